# revision 7
# baseline (speedup 1.0000x reference)
"""DeepSeekMoE (7 routed experts top-3 + 1 shared expert) on 8 NeuronCores.

Strategy: expert-parallel with host-side dispatch.
  - Host computes the (cheap) routing: softmax gate over 7 experts, top-3.
  - Cores 0-6 each own one routed expert's weights (slot A, capacity 4096
    tokens) plus a 512-token chunk of the shared expert (slot B).
    Core 7 runs the shared expert on its remaining 4608 tokens.
    8192 shared tokens = 7*512 + 4608; ~3500 routed tokens per expert.
  - Device kernel (same NEFF on all 8 cores) computes
        Y = (silu(X @ W1) * (X @ Wg)) @ W2 * scale
    for its 4608 gathered tokens in bf16 (f32 PSUM accumulation), where
    `scale` is the per-token combine weight (normalized top-k gate proba for
    routed slots, 1.0 for shared slots).
  - Host scatter-adds the per-core outputs into the full [B,S,D] result.
"""

import threading

import numpy as np
import ml_dtypes

import concourse.bacc as bacc
import concourse.mybir as mybir
import concourse.tile as tile
from concourse.bass_utils import run_bass_kernel_spmd

BF16 = mybir.dt.bfloat16
F32 = mybir.dt.float32
NP_BF16 = ml_dtypes.bfloat16

B, S, D, H = 4, 2048, 2048, 2048
E, TOPK = 7, 3
NTOK = B * S                  # 8192 tokens
T_A, T_B = 4096, 512          # per-core slot capacities
T = T_A + T_B                 # 4608 tokens per core
BLK = 512                     # token block (matmul moving free dim)
NBLK = T // BLK               # 9
KT = D // 128                 # 16 contraction k-tiles for GEMM1
HKT = H // 128                # 16 contraction k-tiles for GEMM2
N_CORES = 8

TRACE = False                 # set by test harness to capture a profile
LAST_RESULT = None            # BassKernelResults of the last run

_nc_cache = None
_nc_lock = threading.Lock()


def _build_nc():
    """Build + schedule the per-core Bass module (one NEFF, SPMD on 8 cores)."""
    nc = bacc.Bacc("TRN2", target_bir_lowering=False, debug=False,
                   num_devices=N_CORES)

    xt = nc.dram_tensor("xt", [D, T], BF16, kind="ExternalInput")
    w1a = nc.dram_tensor("w1a", [D, H], BF16, kind="ExternalInput")
    wga = nc.dram_tensor("wga", [D, H], BF16, kind="ExternalInput")
    w2a = nc.dram_tensor("w2a", [H, D], BF16, kind="ExternalInput")
    w1b = nc.dram_tensor("w1b", [D, H], BF16, kind="ExternalInput")
    wgb = nc.dram_tensor("wgb", [D, H], BF16, kind="ExternalInput")
    w2b = nc.dram_tensor("w2b", [H, D], BF16, kind="ExternalInput")
    sc = nc.dram_tensor("sc", [128, T // 128], F32, kind="ExternalInput")
    y = nc.dram_tensor("y", [T, D], F32, kind="ExternalOutput")

    slots_1 = [(w1a, wga, 0, T_A // BLK), (w1b, wgb, T_A // BLK, T_B // BLK)]
    slots_2 = [(w2a, 0, T_A // BLK), (w2b, T_A // BLK, T_B // BLK)]

    with tile.TileContext(nc) as tc:
        with tc.tile_pool(name="dram", bufs=1, space="DRAM") as dpool:
            ht_dram = dpool.tile([H, T], BF16)

            # ---- Phase 1: HT[h, t] = silu(x@W1).T * (x@Wg).T  (bf16) ----
            with tc.tile_pool(name="w1p", bufs=1) as wpool, \
                 tc.tile_pool(name="xp", bufs=2) as xpool, \
                 tc.tile_pool(name="hp", bufs=3) as hpool, \
                 tc.tile_pool(name="ps1", bufs=2, space="PSUM") as pspool:
                for w1_d, wg_d, blk0, nblk in slots_1:
                    w1_sb = wpool.tile([128, KT, H], BF16, tag="w1")
                    wg_sb = wpool.tile([128, KT, H], BF16, tag="wg")
                    w1_r = w1_d.rearrange("(ko p) h -> p ko h", p=128)
                    wg_r = wg_d.rearrange("(ko p) h -> p ko h", p=128)
                    for k in range(KT):
                        nc.sync.dma_start(w1_sb[:, k], w1_r[:, k])
                        nc.sync.dma_start(wg_sb[:, k], wg_r[:, k])
                    for tb in range(blk0, blk0 + nblk):
                        c0 = tb * BLK
                        xt_sb = xpool.tile([128, KT, BLK], BF16, tag="xt")
                        xt_r = xt[:, c0:c0 + BLK].rearrange(
                            "(ko p) t -> p ko t", p=128)
                        nc.sync.dma_start(xt_sb[:], xt_r)
                        for h in range(H // 128):
                            hs = slice(h * 128, (h + 1) * 128)
                            ps_1 = pspool.tile([128, BLK], F32, tag="ps1")
                            for k in range(KT):
                                nc.tensor.matmul(
                                    ps_1, w1_sb[:, k, hs], xt_sb[:, k],
                                    start=(k == 0), stop=(k == KT - 1))
                            ps_g = pspool.tile([128, BLK], F32, tag="psg")
                            for k in range(KT):
                                nc.tensor.matmul(
                                    ps_g, wg_sb[:, k, hs], xt_sb[:, k],
                                    start=(k == 0), stop=(k == KT - 1))
                            sil = hpool.tile([128, BLK], BF16, tag="sil")
                            nc.scalar.activation(
                                sil[:], ps_1[:],
                                mybir.ActivationFunctionType.Silu)
                            hpt = hpool.tile([128, BLK], BF16, tag="ht")
                            nc.vector.tensor_tensor(
                                hpt[:], sil[:], ps_g[:], mybir.AluOpType.mult)
                            nc.sync.dma_start(
                                ht_dram[hs, c0:c0 + BLK], hpt[:])

            # ---- Phase 2: Y[t, d] = (HT.T @ W2) * scale[t]  (f32 out) ----
            with tc.tile_pool(name="w2p", bufs=2) as w2pool, \
                 tc.tile_pool(name="hp2", bufs=2) as hpool2, \
                 tc.tile_pool(name="scp", bufs=1) as scpool, \
                 tc.tile_pool(name="yp", bufs=3) as ypool, \
                 tc.tile_pool(name="ps2", bufs=4, space="PSUM") as pspool2:
                sc_sb = scpool.tile([128, T // 128], F32)
                nc.sync.dma_start(sc_sb[:], sc[:, :])
                for w2_d, blk0, nblk in slots_2:
                    w2_sb = w2pool.tile([128, HKT, D], BF16, tag="w2")
                    w2_r = w2_d.rearrange("(ko p) d -> p ko d", p=128)
                    for k in range(HKT):
                        nc.sync.dma_start(w2_sb[:, k], w2_r[:, k])
                    for tb in range(blk0, blk0 + nblk):
                        c0 = tb * BLK
                        ht_sb = hpool2.tile([128, HKT, BLK], BF16, tag="ht2")
                        ht_r = ht_dram[:, c0:c0 + BLK].rearrange(
                            "(ko p) t -> p ko t", p=128)
                        nc.sync.dma_start(ht_sb[:], ht_r)
                        for i in range(BLK // 128):      # token subtiles
                            ts_ = slice(i * 128, (i + 1) * 128)
                            for j in range(D // 512):    # output d subtiles
                                ds_ = slice(j * 512, (j + 1) * 512)
                                psy = pspool2.tile([128, 512], F32, tag="psy")
                                for k in range(HKT):
                                    nc.tensor.matmul(
                                        psy, ht_sb[:, k, ts_],
                                        w2_sb[:, k, ds_],
                                        start=(k == 0), stop=(k == HKT - 1))
                                yt_sb = ypool.tile([128, 512], F32, tag="y")
                                col = tb * 4 + i
                                nc.vector.tensor_scalar_mul(
                                    yt_sb[:], psy[:], sc_sb[:, col:col + 1])
                                nc.sync.dma_start(
                                    y[c0 + i * 128:c0 + (i + 1) * 128, ds_],
                                    yt_sb[:])
    nc.compile()
    return nc


def _get_nc():
    global _nc_cache
    with _nc_lock:
        if _nc_cache is None:
            _nc_cache = _build_nc()
        return _nc_cache


def benchmark(in_maps, iters=8, inner=1):
    """Time the NEFF execution with device-resident inputs.

    Chains `inner` sequential executions inside one jitted call (each
    round's outputs feed the next round's output-seed operands, forcing
    serialization), so per-exec time excludes host<->device transfer and
    most dispatch overhead. Returns (best_per_exec_seconds, outputs_list).
    """
    import time as _time

    import jax
    from jax.sharding import Mesh, NamedSharding, PartitionSpec
    from jax.experimental.shard_map import shard_map

    from concourse import bass2jax, mybir as _mybir

    nc = _get_nc()
    bass2jax.install_neuronx_cc_hook()

    partition_name = (nc.partition_id_tensor.name
                      if nc.partition_id_tensor else None)
    in_names, out_names, out_avals, zero_outs = [], [], [], []
    for alloc in nc.m.functions[0].allocations:
        if not isinstance(alloc, _mybir.MemoryLocationSet):
            continue
        name = alloc.memorylocations[0].name
        if alloc.kind == "ExternalInput":
            if name != partition_name:
                in_names.append(name)
        elif alloc.kind == "ExternalOutput":
            out_names.append(name)
            shape = tuple(alloc.tensor_shape)
            dtype = _mybir.dt.np(alloc.dtype)
            out_avals.append(jax.core.ShapedArray(shape, dtype))
            zero_outs.append(np.zeros(shape, dtype))
    n_params = len(in_names)
    all_names = in_names + out_names
    if partition_name is not None:
        all_names = all_names + [partition_name]

    def _exec_once(args, outs):
        extra = ([bass2jax.partition_id_tensor()]
                 if partition_name is not None else [])
        return bass2jax._bass_exec_p.bind(
            *args, *outs, *extra,
            out_avals=tuple(out_avals),
            in_names=tuple(all_names),
            out_names=tuple(out_names),
            lowering_input_output_aliases=(),
            sim_require_finite=True,
            sim_require_nnan=True,
            nc=nc,
        )

    def _body(*args):
        ins, outs = args[:n_params], list(args[n_params:])
        for _ in range(inner):
            outs = list(_exec_once(ins, outs))
        return tuple(outs)

    n_cores = len(in_maps)
    devices = jax.devices()[:n_cores]
    mesh = Mesh(np.asarray(devices), ("core",))
    spec = PartitionSpec("core")
    sharded = jax.jit(
        shard_map(_body, mesh=mesh,
                  in_specs=(spec,) * (n_params + len(out_names)),
                  out_specs=(spec,) * len(out_names), check_rep=False),
        keep_unused=True)

    sh = NamedSharding(mesh, spec)
    dev_in = [
        jax.device_put(
            np.concatenate([np.asarray(in_maps[c][nm]) for c in range(n_cores)],
                           axis=0), sh)
        for nm in in_names
    ]
    dev_zero = [
        jax.device_put(np.zeros((n_cores * z.shape[0], *z.shape[1:]), z.dtype),
                       sh)
        for z in zero_outs
    ]
    out = sharded(*dev_in, *dev_zero)
    jax.block_until_ready(out)

    best = float("inf")
    for _ in range(iters):
        t0 = _time.perf_counter()
        out = sharded(*dev_in, *dev_zero)
        jax.block_until_ready(out)
        best = min(best, (_time.perf_counter() - t0) / inner)

    results = [
        {nm: np.asarray(out[i]).reshape(n_cores, *out_avals[i].shape)[c]
         for i, nm in enumerate(out_names)}
        for c in range(n_cores)
    ]
    return best, results


def _softmax_f32(x):
    m = x.max(axis=-1, keepdims=True)
    e = np.exp((x - m).astype(np.float64))
    return (e / e.sum(axis=-1, keepdims=True)).astype(np.float32)


def _np_ffn(x, w1, wg, w2):
    h1 = x @ w1
    return ((h1 / (1.0 + np.exp(-h1))) * (x @ wg)) @ w2


def _dispatch(x, W1, Wg, W2, Ws1, Wsg, Ws2, gate_w, gate_b, biases):
    """Host-side routing + sharding. Returns (in_maps, core_idx, overflow, xf)."""
    x = np.asarray(x, dtype=np.float32)
    W1 = np.asarray(W1, dtype=np.float32)
    Wg = np.asarray(Wg, dtype=np.float32)
    W2 = np.asarray(W2, dtype=np.float32)
    Ws1 = np.asarray(Ws1, dtype=np.float32)
    Wsg = np.asarray(Wsg, dtype=np.float32)
    Ws2 = np.asarray(Ws2, dtype=np.float32)
    gate_w = np.asarray(gate_w, dtype=np.float32)
    gate_b = np.asarray(gate_b, dtype=np.float32)
    biases = np.asarray(biases, dtype=np.float32)

    xf = x.reshape(NTOK, D)

    # ---- routing (host): mirrors the reference math ----
    logits = xf @ gate_w + gate_b
    probas = _softmax_f32(logits)
    biased = probas + biases
    # jax.lax.top_k tie-break: lowest index first -> stable argsort of -biased
    topk = np.argsort(-biased, axis=-1, kind="stable")[:, :TOPK]
    tp = np.take_along_axis(probas, topk, axis=-1)
    tp = tp / tp.sum(axis=-1, keepdims=True)

    # ---- dispatch ----
    xbf = xf.astype(NP_BF16)
    w1bf = [W1[e].astype(NP_BF16) for e in range(E)]
    wgbf = [Wg[e].astype(NP_BF16) for e in range(E)]
    w2bf = [W2[e].astype(NP_BF16) for e in range(E)]
    ws1bf, wsgbf, ws2bf = (Ws1.astype(NP_BF16), Wsg.astype(NP_BF16),
                           Ws2.astype(NP_BF16))

    expert_tok = []   # token ids routed to expert e
    expert_wt = []    # their combine weights
    overflow = []     # (token, expert, weight) pairs beyond slot capacity
    for e in range(E):
        sel = (topk == e)
        rows = np.where(sel.any(axis=-1))[0]
        wts = (tp * sel).sum(axis=-1)[rows]
        if len(rows) > T_A:
            for t, w in zip(rows[T_A:], wts[T_A:]):
                overflow.append((int(t), e, float(w)))
            rows, wts = rows[:T_A], wts[:T_A]
        expert_tok.append(rows)
        expert_wt.append(wts.astype(np.float32))

    shared_chunks = [np.arange(T_A + T_B + 512 * i, T_A + T_B + 512 * (i + 1))
                     for i in range(E)]          # cores 0-6: 512 tokens each
    shared_chunks.append(np.arange(0, T_A + T_B))  # core 7: 4608 tokens

    in_maps = []
    core_idx = []   # (idxA, nA, idxB, nB) for the combine step
    for c in range(N_CORES):
        if c < E:
            idx_a, wt_a = expert_tok[c], expert_wt[c]
            w1s, wgs, w2s = w1bf[c], wgbf[c], w2bf[c]
            idx_b = shared_chunks[c]
        else:
            idx_a = shared_chunks[c][:T_A]
            wt_a = np.ones(T_A, np.float32)
            w1s, wgs, w2s = ws1bf, wsgbf, ws2bf
            idx_b = shared_chunks[c][T_A:]
        n_a, n_b = len(idx_a), len(idx_b)

        xg = np.zeros((T, D), dtype=NP_BF16)
        xg[:n_a] = xbf[idx_a]
        xg[T_A:T_A + n_b] = xbf[idx_b]
        xt_c = np.ascontiguousarray(xg.T)

        s = np.zeros(T, np.float32)
        s[:n_a] = wt_a
        s[T_A:T_A + n_b] = 1.0
        sc_c = np.ascontiguousarray(s.reshape(T // 128, 128).T)

        in_maps.append({
            "xt": xt_c, "sc": sc_c,
            "w1a": w1s, "wga": wgs, "w2a": w2s,
            "w1b": ws1bf, "wgb": wsgbf, "w2b": ws2bf,
        })
        core_idx.append((idx_a, n_a, idx_b, n_b))

    return in_maps, core_idx, overflow, xf


def _combine(results, core_idx, overflow, xf, W1, Wg, W2):
    out = np.zeros((NTOK, D), np.float32)
    for c in range(N_CORES):
        yc = results[c]["y"]
        idx_a, n_a, idx_b, n_b = core_idx[c]
        out[idx_a] += yc[:n_a]
        out[idx_b] += yc[T_A:T_A + n_b]

    # correctness fallback if an expert exceeded slot capacity (never happens
    # for the benchmark distribution, but keeps the kernel total-correct)
    for t, e, w in overflow:
        out[t] += w * _np_ffn(xf[t:t + 1], np.asarray(W1[e], np.float32),
                              np.asarray(Wg[e], np.float32),
                              np.asarray(W2[e], np.float32))[0]

    return out.reshape(B, S, D)


def kernel(x, W1, Wg, W2, Ws1, Wsg, Ws2, gate_w, gate_b, biases):
    global LAST_RESULT
    in_maps, core_idx, overflow, xf = _dispatch(
        x, W1, Wg, W2, Ws1, Wsg, Ws2, gate_w, gate_b, biases)

    nc = _get_nc()
    res = run_bass_kernel_spmd(nc, in_maps, core_ids=list(range(N_CORES)))
    LAST_RESULT = res

    return _combine(res.results, core_idx, overflow, xf, W1, Wg, W2)


# revision 22
# speedup vs baseline: 24.2009x; 24.2009x over previous
"""DeepSeekMoE (7 routed experts top-3 + 1 shared expert) on 8 NeuronCores.

Strategy: expert-parallel with host-side dispatch.
  - Host computes the (cheap) routing: softmax gate over 7 experts, top-3.
  - Cores 0-6 each own one routed expert's weights (slot A, capacity 4096
    tokens) plus a 512-token chunk of the shared expert (slot B).
    Core 7 runs the shared expert on its remaining 4608 tokens.
    8192 shared tokens = 7*512 + 4608; ~3500 routed tokens per expert.
  - Device kernel (same NEFF on all 8 cores) computes
        Y = (silu(X @ W1) * (X @ Wg)) @ W2 * scale
    for its 4608 gathered tokens in bf16 (f32 PSUM accumulation), where
    `scale` is the per-token combine weight (normalized top-k gate proba for
    routed slots, 1.0 for shared slots).
  - Host scatter-adds the per-core outputs into the full [B,S,D] result.
"""

import threading

import numpy as np
import ml_dtypes

import concourse.bacc as bacc
import concourse.mybir as mybir
import concourse.tile as tile
from concourse.bass_utils import run_bass_kernel_spmd

BF16 = mybir.dt.bfloat16
F32 = mybir.dt.float32
NP_BF16 = ml_dtypes.bfloat16

B, S, D, H = 4, 2048, 2048, 2048
E, TOPK = 7, 3
NTOK = B * S                  # 8192 tokens
T_A, T_B = 3584, 576          # per-core slot capacities (see below)
T = T_A + T_B                 # 4160 tokens per core
SC_COLS = (T + 127) // 128    # 33 columns of the per-token scale tile
KT = D // 128                 # 16 contraction k-tiles for GEMM1
HKT = H // 128                # 16 contraction k-tiles for GEMM2
N_CORES = 8
# token blocks: (column offset, width, slot) — slot A: 7x512, slot B: 512+64.
# T_A >= max tokens per routed expert (3542 for the benchmark inputs);
# T_A + 8*T_B == NTOK so the shared expert is covered exactly:
# core 7 runs shared on all its 4160 tokens, cores 0-6 on 576 each.
BLOCKS = [(i * 512, 512, 0) for i in range(7)] + [(3584, 512, 1), (4096, 64, 1)]

TRACE = False                 # set by test harness to capture a profile
LAST_RESULT = None            # BassKernelResults of the last run

_nc_cache = {}
_nc_lock = threading.Lock()


def _build_nc(loop_k=1):
    """Build + schedule the per-core Bass module (one NEFF, SPMD on 8 cores).

    loop_k > 1 wraps each phase in a hardware For_i loop that repeats the
    (idempotent) body loop_k times — used only for on-device timing.
    """
    import contextlib

    nc = bacc.Bacc("TRN2", target_bir_lowering=False, debug=False,
                   num_devices=N_CORES)

    xt = nc.dram_tensor("xt", [D, T], BF16, kind="ExternalInput")
    w1a = nc.dram_tensor("w1a", [D, H], BF16, kind="ExternalInput")
    wga = nc.dram_tensor("wga", [D, H], BF16, kind="ExternalInput")
    w2a = nc.dram_tensor("w2a", [H, D], BF16, kind="ExternalInput")
    w1b = nc.dram_tensor("w1b", [D, H], BF16, kind="ExternalInput")
    wgb = nc.dram_tensor("wgb", [D, H], BF16, kind="ExternalInput")
    w2b = nc.dram_tensor("w2b", [H, D], BF16, kind="ExternalInput")
    sc = nc.dram_tensor("sc", [128, SC_COLS], F32, kind="ExternalInput")
    y = nc.dram_tensor("y", [T, D], F32, kind="ExternalOutput")

    slots_1 = [(w1a, wga, 0), (w1b, wgb, 1)]
    slots_2 = [(w2a, 0), (w2b, 1)]

    with tile.TileContext(nc) as tc:
        with tc.tile_pool(name="dram", bufs=1, space="DRAM") as dpool:
            ht_dram = dpool.tile([H, T], BF16)

            # ---- Phase 1: HT[h, t] = silu(x@W1).T * (x@Wg).T  (bf16) ----
            with tc.tile_pool(name="w1p", bufs=1) as wpool, \
                 tc.tile_pool(name="xp", bufs=2) as xpool, \
                 tc.tile_pool(name="hp", bufs=3) as hpool, \
                 tc.tile_pool(name="ps1", bufs=2, space="PSUM") as pspool, \
                 (tc.For_i(0, loop_k, 1) if loop_k > 1
                  else contextlib.nullcontext()):
                for w1_d, wg_d, slot in slots_1:
                    w1_sb = wpool.tile([128, KT, H], BF16, tag="w1")
                    wg_sb = wpool.tile([128, KT, H], BF16, tag="wg")
                    w1_r = w1_d.rearrange("(ko p) h -> p ko h", p=128)
                    wg_r = wg_d.rearrange("(ko p) h -> p ko h", p=128)
                    # split by output column block so h=0 matmuls can start
                    # as soon as their weight slice lands (hides reloads)
                    for h in range(H // 128):
                        hs = slice(h * 128, (h + 1) * 128)
                        nc.sync.dma_start(w1_sb[:, :, hs], w1_r[:, :, hs])
                        nc.sync.dma_start(wg_sb[:, :, hs], wg_r[:, :, hs])
                    for c0, bw, bslot in BLOCKS:
                        if bslot != slot:
                            continue
                        xt_sb = xpool.tile([128, KT, 512], BF16, tag="xt")
                        xt_r = xt[:, c0:c0 + bw].rearrange(
                            "(ko p) t -> p ko t", p=128)
                        nc.sync.dma_start(xt_sb[:, :, :bw], xt_r)
                        for h in range(H // 128):
                            hs = slice(h * 128, (h + 1) * 128)
                            ps_1 = pspool.tile([128, 512], F32, tag="ps1")
                            for k in range(KT):
                                nc.tensor.matmul(
                                    ps_1[:, :bw], w1_sb[:, k, hs],
                                    xt_sb[:, k, :bw],
                                    start=(k == 0), stop=(k == KT - 1))
                            ps_g = pspool.tile([128, 512], F32, tag="psg")
                            for k in range(KT):
                                nc.tensor.matmul(
                                    ps_g[:, :bw], wg_sb[:, k, hs],
                                    xt_sb[:, k, :bw],
                                    start=(k == 0), stop=(k == KT - 1))
                            sil = hpool.tile([128, 512], BF16, tag="sil")
                            nc.scalar.activation(
                                sil[:, :bw], ps_1[:, :bw],
                                mybir.ActivationFunctionType.Silu)
                            hpt = hpool.tile([128, 512], BF16, tag="ht")
                            nc.vector.tensor_tensor(
                                hpt[:, :bw], sil[:, :bw], ps_g[:, :bw],
                                mybir.AluOpType.mult)
                            nc.sync.dma_start(
                                ht_dram[hs, c0:c0 + bw], hpt[:, :bw])

            # ---- Phase 2: Y[t, d] = (HT.T @ W2) * scale[t]  (f32 out) ----
            with tc.tile_pool(name="w2p", bufs=2) as w2pool, \
                 tc.tile_pool(name="hp2", bufs=2) as hpool2, \
                 tc.tile_pool(name="scp", bufs=1) as scpool, \
                 tc.tile_pool(name="yp", bufs=3) as ypool, \
                 tc.tile_pool(name="ps2", bufs=4, space="PSUM") as pspool2, \
                 (tc.For_i(0, loop_k, 1) if loop_k > 1
                  else contextlib.nullcontext()):
                sc_sb = scpool.tile([128, SC_COLS], F32)
                nc.sync.dma_start(sc_sb[:], sc[:, :])
                for w2_d, slot in slots_2:
                    w2_sb = w2pool.tile([128, HKT, D], BF16, tag="w2")
                    w2_r = w2_d.rearrange("(ko p) d -> p ko d", p=128)
                    for j in range(D // 512):
                        ds_ = slice(j * 512, (j + 1) * 512)
                        nc.sync.dma_start(w2_sb[:, :, ds_], w2_r[:, :, ds_])
                    for c0, bw, bslot in BLOCKS:
                        if bslot != slot:
                            continue
                        ht_sb = hpool2.tile([128, HKT, 512], BF16, tag="ht2")
                        ht_r = ht_dram[:, c0:c0 + bw].rearrange(
                            "(ko p) t -> p ko t", p=128)
                        nc.sync.dma_start(ht_sb[:, :, :bw], ht_r)
                        for i in range((bw + 127) // 128):  # token subtiles
                            tw = min(128, bw - i * 128)
                            ts_ = slice(i * 128, i * 128 + tw)
                            for j in range(D // 512):       # output d subtiles
                                ds_ = slice(j * 512, (j + 1) * 512)
                                psy = pspool2.tile([128, 512], F32, tag="psy")
                                for k in range(HKT):
                                    nc.tensor.matmul(
                                        psy[:tw], ht_sb[:, k, ts_],
                                        w2_sb[:, k, ds_],
                                        start=(k == 0), stop=(k == HKT - 1))
                                yt_sb = ypool.tile([128, 512], F32, tag="y")
                                col = (c0 + i * 128) // 128
                                nc.vector.tensor_scalar_mul(
                                    yt_sb[:tw], psy[:tw],
                                    sc_sb[:tw, col:col + 1])
                                nc.sync.dma_start(
                                    y[c0 + i * 128:c0 + i * 128 + tw, ds_],
                                    yt_sb[:tw])
    nc.compile()
    return nc


def _get_nc(loop_k=1):
    with _nc_lock:
        if loop_k not in _nc_cache:
            _nc_cache[loop_k] = _build_nc(loop_k)
        return _nc_cache[loop_k]


def benchmark(in_maps, iters=8, loop_k=1):
    """Time the NEFF execution with device-resident inputs.

    Returns (best_wall_seconds_per_call, outputs_list). With loop_k > 1 the
    NEFF repeats the kernel body loop_k times on-device; comparing against
    loop_k=1 cancels the (large, ~100ms) axon dispatch overhead.
    """
    import time as _time

    import jax
    from jax.sharding import Mesh, NamedSharding, PartitionSpec
    from jax.experimental.shard_map import shard_map

    from concourse import bass2jax, mybir as _mybir

    nc = _get_nc(loop_k)
    bass2jax.install_neuronx_cc_hook()

    partition_name = (nc.partition_id_tensor.name
                      if nc.partition_id_tensor else None)
    in_names, out_names, out_avals, zero_outs = [], [], [], []
    for alloc in nc.m.functions[0].allocations:
        if not isinstance(alloc, _mybir.MemoryLocationSet):
            continue
        name = alloc.memorylocations[0].name
        if alloc.kind == "ExternalInput":
            if name != partition_name:
                in_names.append(name)
        elif alloc.kind == "ExternalOutput":
            out_names.append(name)
            shape = tuple(alloc.tensor_shape)
            dtype = _mybir.dt.np(alloc.dtype)
            out_avals.append(jax.core.ShapedArray(shape, dtype))
            zero_outs.append(np.zeros(shape, dtype))
    n_params = len(in_names)
    all_names = in_names + out_names
    if partition_name is not None:
        all_names = all_names + [partition_name]

    def _exec_once(args, outs):
        extra = ([bass2jax.partition_id_tensor()]
                 if partition_name is not None else [])
        return bass2jax._bass_exec_p.bind(
            *args, *outs, *extra,
            out_avals=tuple(out_avals),
            in_names=tuple(all_names),
            out_names=tuple(out_names),
            lowering_input_output_aliases=(),
            sim_require_finite=True,
            sim_require_nnan=True,
            nc=nc,
        )

    def _body(*args):
        ins, outs = args[:n_params], list(args[n_params:])
        return tuple(_exec_once(ins, outs))

    n_cores = len(in_maps)
    devices = jax.devices()[:n_cores]
    mesh = Mesh(np.asarray(devices), ("core",))
    spec = PartitionSpec("core")
    sharded = jax.jit(
        shard_map(_body, mesh=mesh,
                  in_specs=(spec,) * (n_params + len(out_names)),
                  out_specs=(spec,) * len(out_names), check_rep=False),
        keep_unused=True)

    sh = NamedSharding(mesh, spec)
    dev_in = [
        jax.device_put(
            np.concatenate([np.asarray(in_maps[c][nm]) for c in range(n_cores)],
                           axis=0), sh)
        for nm in in_names
    ]
    dev_zero = [
        jax.device_put(np.zeros((n_cores * z.shape[0], *z.shape[1:]), z.dtype),
                       sh)
        for z in zero_outs
    ]
    out = sharded(*dev_in, *dev_zero)
    jax.block_until_ready(out)

    best = float("inf")
    for _ in range(iters):
        t0 = _time.perf_counter()
        out = sharded(*dev_in, *dev_zero)
        jax.block_until_ready(out)
        best = min(best, _time.perf_counter() - t0)

    results = [
        {nm: np.asarray(out[i]).reshape(n_cores, *out_avals[i].shape)[c]
         for i, nm in enumerate(out_names)}
        for c in range(n_cores)
    ]
    return best, results


def _softmax_f32(x):
    m = x.max(axis=-1, keepdims=True)
    e = np.exp((x - m).astype(np.float64))
    return (e / e.sum(axis=-1, keepdims=True)).astype(np.float32)


def _np_ffn(x, w1, wg, w2):
    h1 = x @ w1
    return ((h1 / (1.0 + np.exp(-h1))) * (x @ wg)) @ w2


def _dispatch(x, W1, Wg, W2, Ws1, Wsg, Ws2, gate_w, gate_b, biases):
    """Host-side routing + sharding. Returns (in_maps, core_idx, overflow, xf)."""
    x = np.asarray(x, dtype=np.float32)
    W1 = np.asarray(W1, dtype=np.float32)
    Wg = np.asarray(Wg, dtype=np.float32)
    W2 = np.asarray(W2, dtype=np.float32)
    Ws1 = np.asarray(Ws1, dtype=np.float32)
    Wsg = np.asarray(Wsg, dtype=np.float32)
    Ws2 = np.asarray(Ws2, dtype=np.float32)
    gate_w = np.asarray(gate_w, dtype=np.float32)
    gate_b = np.asarray(gate_b, dtype=np.float32)
    biases = np.asarray(biases, dtype=np.float32)

    xf = x.reshape(NTOK, D)

    # ---- routing (host): mirrors the reference math ----
    logits = xf @ gate_w + gate_b
    probas = _softmax_f32(logits)
    biased = probas + biases
    # jax.lax.top_k tie-break: lowest index first -> stable argsort of -biased
    topk = np.argsort(-biased, axis=-1, kind="stable")[:, :TOPK]
    tp = np.take_along_axis(probas, topk, axis=-1)
    tp = tp / tp.sum(axis=-1, keepdims=True)

    # ---- dispatch ----
    xbf = xf.astype(NP_BF16)
    w1bf = [W1[e].astype(NP_BF16) for e in range(E)]
    wgbf = [Wg[e].astype(NP_BF16) for e in range(E)]
    w2bf = [W2[e].astype(NP_BF16) for e in range(E)]
    ws1bf, wsgbf, ws2bf = (Ws1.astype(NP_BF16), Wsg.astype(NP_BF16),
                           Ws2.astype(NP_BF16))

    expert_tok = []   # token ids routed to expert e
    expert_wt = []    # their combine weights
    overflow = []     # (token, expert, weight) pairs beyond slot capacity
    for e in range(E):
        sel = (topk == e)
        rows = np.where(sel.any(axis=-1))[0]
        wts = (tp * sel).sum(axis=-1)[rows]
        if len(rows) > T_A:
            for t, w in zip(rows[T_A:], wts[T_A:]):
                overflow.append((int(t), e, float(w)))
            rows, wts = rows[:T_A], wts[:T_A]
        expert_tok.append(rows)
        expert_wt.append(wts.astype(np.float32))

    shared_chunks = [np.arange(T + T_B * i, T + T_B * (i + 1))
                     for i in range(E)]          # cores 0-6: T_B tokens each
    shared_chunks.append(np.arange(0, T))        # core 7: T tokens

    in_maps = []
    core_idx = []   # (idxA, nA, idxB, nB) for the combine step
    for c in range(N_CORES):
        if c < E:
            idx_a, wt_a = expert_tok[c], expert_wt[c]
            w1s, wgs, w2s = w1bf[c], wgbf[c], w2bf[c]
            idx_b = shared_chunks[c]
        else:
            idx_a = shared_chunks[c][:T_A]
            wt_a = np.ones(T_A, np.float32)
            w1s, wgs, w2s = ws1bf, wsgbf, ws2bf
            idx_b = shared_chunks[c][T_A:]
        n_a, n_b = len(idx_a), len(idx_b)

        xg = np.zeros((T, D), dtype=NP_BF16)
        xg[:n_a] = xbf[idx_a]
        xg[T_A:T_A + n_b] = xbf[idx_b]
        xt_c = np.ascontiguousarray(xg.T)

        s = np.zeros(SC_COLS * 128, np.float32)
        s[:n_a] = wt_a
        s[T_A:T_A + n_b] = 1.0
        sc_c = np.ascontiguousarray(s.reshape(SC_COLS, 128).T)

        in_maps.append({
            "xt": xt_c, "sc": sc_c,
            "w1a": w1s, "wga": wgs, "w2a": w2s,
            "w1b": ws1bf, "wgb": wsgbf, "w2b": ws2bf,
        })
        core_idx.append((idx_a, n_a, idx_b, n_b))

    return in_maps, core_idx, overflow, xf


def _combine(results, core_idx, overflow, xf, W1, Wg, W2):
    out = np.zeros((NTOK, D), np.float32)
    for c in range(N_CORES):
        yc = results[c]["y"]
        idx_a, n_a, idx_b, n_b = core_idx[c]
        out[idx_a] += yc[:n_a]
        out[idx_b] += yc[T_A:T_A + n_b]

    # correctness fallback if an expert exceeded slot capacity (never happens
    # for the benchmark distribution, but keeps the kernel total-correct)
    for t, e, w in overflow:
        out[t] += w * _np_ffn(xf[t:t + 1], np.asarray(W1[e], np.float32),
                              np.asarray(Wg[e], np.float32),
                              np.asarray(W2[e], np.float32))[0]

    return out.reshape(B, S, D)


def kernel(x, W1, Wg, W2, Ws1, Wsg, Ws2, gate_w, gate_b, biases):
    global LAST_RESULT
    in_maps, core_idx, overflow, xf = _dispatch(
        x, W1, Wg, W2, Ws1, Wsg, Ws2, gate_w, gate_b, biases)

    nc = _get_nc()
    res = run_bass_kernel_spmd(nc, in_maps, core_ids=list(range(N_CORES)))
    LAST_RESULT = res

    return _combine(res.results, core_idx, overflow, xf, W1, Wg, W2)


# revision 28
# speedup vs baseline: 32.3132x; 1.3352x over previous
"""DeepSeekMoE (7 routed experts top-3 + 1 shared expert) on 8 NeuronCores.

Strategy: expert-parallel with host-side dispatch.
  - Host computes the (cheap) routing: softmax gate over 7 experts, top-3.
  - Cores 0-6 each own one routed expert's weights (slot A, capacity 3584
    tokens; ~3500 tokens route to each expert) plus a 576-token chunk of the
    shared expert (slot B). Core 7 runs the shared expert on its remaining
    4160 tokens. 8192 shared tokens = 7*576 + 4160.
  - Device kernel (same NEFF on all 8 cores) computes
        Y = (silu(X @ W1) * (X @ Wg)) @ W2 * scale
    for its 4160 gathered tokens in bf16 (f32 PSUM accumulation), where
    `scale` is the per-token combine weight (normalized top-k gate proba for
    routed slots, 1.0 for shared slots).
  - Host scatter-adds the per-core outputs into the full [B,S,D] result.
"""

import threading

import numpy as np
import ml_dtypes

import concourse.bacc as bacc
import concourse.mybir as mybir
import concourse.tile as tile
from concourse.bass_utils import run_bass_kernel_spmd

BF16 = mybir.dt.bfloat16
F32 = mybir.dt.float32
NP_BF16 = ml_dtypes.bfloat16

B, S, D, H = 4, 2048, 2048, 2048
E, TOPK = 7, 3
NTOK = B * S                  # 8192 tokens
T_A, T_B = 3584, 576          # per-core slot capacities (see below)
T = T_A + T_B                 # 4160 tokens per core
SC_COLS = (T + 127) // 128    # 33 columns of the per-token scale tile
KT = D // 128                 # 16 contraction k-tiles for GEMM1
HKT = H // 128                # 16 contraction k-tiles for GEMM2
N_CORES = 8
# token blocks: (column offset, width, slot) — slot A: 7x512, slot B: 512+64.
# T_A >= max tokens per routed expert (3542 for the benchmark inputs);
# T_A + 8*T_B == NTOK so the shared expert is covered exactly:
# core 7 runs shared on all its 4160 tokens, cores 0-6 on 576 each.
BLOCKS = [(i * 512, 512, 0) for i in range(7)] + [(3584, 512, 1), (4096, 64, 1)]

TRACE = False                 # set by test harness to capture a profile
LAST_RESULT = None            # BassKernelResults of the last run
WSPLIT = "kh"                 # weight DMA split: "k" rows only, "kh" k x half

_nc_cache = {}
_nc_lock = threading.Lock()


def _build_nc(loop_k=1):
    """Build + schedule the per-core Bass module (one NEFF, SPMD on 8 cores).

    loop_k > 1 wraps each phase in a hardware For_i loop that repeats the
    (idempotent) body loop_k times — used only for on-device timing.
    """
    import contextlib

    nc = bacc.Bacc("TRN2", target_bir_lowering=False, debug=False,
                   num_devices=N_CORES)

    xt = nc.dram_tensor("xt", [D, T], BF16, kind="ExternalInput")
    w1a = nc.dram_tensor("w1a", [D, H], BF16, kind="ExternalInput")
    wga = nc.dram_tensor("wga", [D, H], BF16, kind="ExternalInput")
    w2a = nc.dram_tensor("w2a", [H, D], BF16, kind="ExternalInput")
    w1b = nc.dram_tensor("w1b", [D, H], BF16, kind="ExternalInput")
    wgb = nc.dram_tensor("wgb", [D, H], BF16, kind="ExternalInput")
    w2b = nc.dram_tensor("w2b", [H, D], BF16, kind="ExternalInput")
    sc = nc.dram_tensor("sc", [128, SC_COLS], F32, kind="ExternalInput")
    y = nc.dram_tensor("y", [T, D], F32, kind="ExternalOutput")

    slots_1 = [(w1a, wga, 0), (w1b, wgb, 1)]
    slots_2 = [(w2a, 0), (w2b, 1)]

    with tile.TileContext(nc) as tc:
        with tc.tile_pool(name="dram", bufs=1, space="DRAM") as dpool:
            ht_dram = dpool.tile([H, T], BF16)

            # ---- Phase 1: HT[h, t] = silu(x@W1).T * (x@Wg).T  (bf16) ----
            with tc.tile_pool(name="w1p", bufs=1) as wpool, \
                 tc.tile_pool(name="xp", bufs=2) as xpool, \
                 tc.tile_pool(name="hp", bufs=3) as hpool, \
                 tc.tile_pool(name="ps1", bufs=2, space="PSUM") as pspool, \
                 (tc.For_i(0, loop_k, 1) if loop_k > 1
                  else contextlib.nullcontext()):
                for w1_d, wg_d, slot in slots_1:
                    w1_sb = wpool.tile([128, KT, H], BF16, tag="w1")
                    wg_sb = wpool.tile([128, KT, H], BF16, tag="wg")
                    w1_r = w1_d.rearrange("(ko p) h -> p ko h", p=128)
                    wg_r = wg_d.rearrange("(ko p) h -> p ko h", p=128)
                    # per-(k, column-half) DMAs: contiguous 2KB lines per
                    # partition (DMA-efficient) while still letting early-h
                    # matmuls start after the first column half lands
                    if WSPLIT == "k":
                        for k in range(KT):
                            nc.sync.dma_start(w1_sb[:, k], w1_r[:, k])
                            nc.sync.dma_start(wg_sb[:, k], wg_r[:, k])
                    else:
                        for hh in range(2):
                            hs = slice(hh * (H // 2), (hh + 1) * (H // 2))
                            for k in range(KT):
                                nc.sync.dma_start(
                                    w1_sb[:, k, hs], w1_r[:, k, hs])
                                nc.sync.dma_start(
                                    wg_sb[:, k, hs], wg_r[:, k, hs])
                    for c0, bw, bslot in BLOCKS:
                        if bslot != slot:
                            continue
                        xt_sb = xpool.tile([128, KT, 512], BF16, tag="xt")
                        xt_r = xt[:, c0:c0 + bw].rearrange(
                            "(ko p) t -> p ko t", p=128)
                        nc.sync.dma_start(xt_sb[:, :, :bw], xt_r)
                        for h in range(H // 128):
                            hs = slice(h * 128, (h + 1) * 128)
                            ps_1 = pspool.tile([128, 512], F32, tag="ps1")
                            for k in range(KT):
                                nc.tensor.matmul(
                                    ps_1[:, :bw], w1_sb[:, k, hs],
                                    xt_sb[:, k, :bw],
                                    start=(k == 0), stop=(k == KT - 1))
                            ps_g = pspool.tile([128, 512], F32, tag="psg")
                            for k in range(KT):
                                nc.tensor.matmul(
                                    ps_g[:, :bw], wg_sb[:, k, hs],
                                    xt_sb[:, k, :bw],
                                    start=(k == 0), stop=(k == KT - 1))
                            sil = hpool.tile([128, 512], BF16, tag="sil")
                            nc.scalar.activation(
                                sil[:, :bw], ps_1[:, :bw],
                                mybir.ActivationFunctionType.Silu)
                            hpt = hpool.tile([128, 512], BF16, tag="ht")
                            nc.vector.tensor_tensor(
                                hpt[:, :bw], sil[:, :bw], ps_g[:, :bw],
                                mybir.AluOpType.mult)
                            nc.sync.dma_start(
                                ht_dram[hs, c0:c0 + bw], hpt[:, :bw])

            # ---- Phase 2: Y[t, d] = (HT.T @ W2) * scale[t]  (f32 out) ----
            with tc.tile_pool(name="w2p", bufs=2) as w2pool, \
                 tc.tile_pool(name="hp2", bufs=2) as hpool2, \
                 tc.tile_pool(name="scp", bufs=1) as scpool, \
                 tc.tile_pool(name="yp", bufs=3) as ypool, \
                 tc.tile_pool(name="ps2", bufs=4, space="PSUM") as pspool2, \
                 (tc.For_i(0, loop_k, 1) if loop_k > 1
                  else contextlib.nullcontext()):
                sc_sb = scpool.tile([128, SC_COLS], F32)
                nc.sync.dma_start(sc_sb[:], sc[:, :])
                for w2_d, slot in slots_2:
                    w2_sb = w2pool.tile([128, HKT, D], BF16, tag="w2")
                    w2_r = w2_d.rearrange("(ko p) d -> p ko d", p=128)
                    if WSPLIT == "k":
                        for k in range(HKT):
                            nc.sync.dma_start(w2_sb[:, k], w2_r[:, k])
                    else:
                        for hh in range(2):
                            ds_ = slice(hh * (D // 2), (hh + 1) * (D // 2))
                            for k in range(HKT):
                                nc.sync.dma_start(
                                    w2_sb[:, k, ds_], w2_r[:, k, ds_])
                    for c0, bw, bslot in BLOCKS:
                        if bslot != slot:
                            continue
                        ht_sb = hpool2.tile([128, HKT, 512], BF16, tag="ht2")
                        ht_r = ht_dram[:, c0:c0 + bw].rearrange(
                            "(ko p) t -> p ko t", p=128)
                        nc.sync.dma_start(ht_sb[:, :, :bw], ht_r)
                        for i in range((bw + 127) // 128):  # token subtiles
                            tw = min(128, bw - i * 128)
                            ts_ = slice(i * 128, i * 128 + tw)
                            for j in range(D // 512):       # output d subtiles
                                ds_ = slice(j * 512, (j + 1) * 512)
                                psy = pspool2.tile([128, 512], F32, tag="psy")
                                for k in range(HKT):
                                    nc.tensor.matmul(
                                        psy[:tw], ht_sb[:, k, ts_],
                                        w2_sb[:, k, ds_],
                                        start=(k == 0), stop=(k == HKT - 1))
                                yt_sb = ypool.tile([128, 512], F32, tag="y")
                                col = (c0 + i * 128) // 128
                                nc.vector.tensor_scalar_mul(
                                    yt_sb[:tw], psy[:tw],
                                    sc_sb[:tw, col:col + 1])
                                nc.sync.dma_start(
                                    y[c0 + i * 128:c0 + i * 128 + tw, ds_],
                                    yt_sb[:tw])
    nc.compile()
    return nc


def _get_nc(loop_k=1):
    with _nc_lock:
        key = (loop_k, WSPLIT)
        if key not in _nc_cache:
            _nc_cache[key] = _build_nc(loop_k)
        return _nc_cache[key]


def benchmark(in_maps, iters=8, loop_k=1):
    """Time the NEFF execution with device-resident inputs.

    Returns (best_wall_seconds_per_call, outputs_list). With loop_k > 1 the
    NEFF repeats the kernel body loop_k times on-device; comparing against
    loop_k=1 cancels the (large, ~100ms) axon dispatch overhead.
    """
    import time as _time

    import jax
    from jax.sharding import Mesh, NamedSharding, PartitionSpec
    from jax.experimental.shard_map import shard_map

    from concourse import bass2jax, mybir as _mybir

    nc = _get_nc(loop_k)
    bass2jax.install_neuronx_cc_hook()

    partition_name = (nc.partition_id_tensor.name
                      if nc.partition_id_tensor else None)
    in_names, out_names, out_avals, zero_outs = [], [], [], []
    for alloc in nc.m.functions[0].allocations:
        if not isinstance(alloc, _mybir.MemoryLocationSet):
            continue
        name = alloc.memorylocations[0].name
        if alloc.kind == "ExternalInput":
            if name != partition_name:
                in_names.append(name)
        elif alloc.kind == "ExternalOutput":
            out_names.append(name)
            shape = tuple(alloc.tensor_shape)
            dtype = _mybir.dt.np(alloc.dtype)
            out_avals.append(jax.core.ShapedArray(shape, dtype))
            zero_outs.append(np.zeros(shape, dtype))
    n_params = len(in_names)
    all_names = in_names + out_names
    if partition_name is not None:
        all_names = all_names + [partition_name]

    def _exec_once(args, outs):
        extra = ([bass2jax.partition_id_tensor()]
                 if partition_name is not None else [])
        return bass2jax._bass_exec_p.bind(
            *args, *outs, *extra,
            out_avals=tuple(out_avals),
            in_names=tuple(all_names),
            out_names=tuple(out_names),
            lowering_input_output_aliases=(),
            sim_require_finite=True,
            sim_require_nnan=True,
            nc=nc,
        )

    def _body(*args):
        ins, outs = args[:n_params], list(args[n_params:])
        return tuple(_exec_once(ins, outs))

    n_cores = len(in_maps)
    devices = jax.devices()[:n_cores]
    mesh = Mesh(np.asarray(devices), ("core",))
    spec = PartitionSpec("core")
    sharded = jax.jit(
        shard_map(_body, mesh=mesh,
                  in_specs=(spec,) * (n_params + len(out_names)),
                  out_specs=(spec,) * len(out_names), check_rep=False),
        keep_unused=True)

    sh = NamedSharding(mesh, spec)
    dev_in = [
        jax.device_put(
            np.concatenate([np.asarray(in_maps[c][nm]) for c in range(n_cores)],
                           axis=0), sh)
        for nm in in_names
    ]
    dev_zero = [
        jax.device_put(np.zeros((n_cores * z.shape[0], *z.shape[1:]), z.dtype),
                       sh)
        for z in zero_outs
    ]
    out = sharded(*dev_in, *dev_zero)
    jax.block_until_ready(out)

    all_times = []
    for _ in range(iters):
        t0 = _time.perf_counter()
        out = sharded(*dev_in, *dev_zero)
        jax.block_until_ready(out)
        all_times.append(_time.perf_counter() - t0)
    best = min(all_times)
    benchmark.last_times = all_times

    results = [
        {nm: np.asarray(out[i]).reshape(n_cores, *out_avals[i].shape)[c]
         for i, nm in enumerate(out_names)}
        for c in range(n_cores)
    ]
    return best, results


def _softmax_f32(x):
    m = x.max(axis=-1, keepdims=True)
    e = np.exp((x - m).astype(np.float64))
    return (e / e.sum(axis=-1, keepdims=True)).astype(np.float32)


def _np_ffn(x, w1, wg, w2):
    h1 = x @ w1
    return ((h1 / (1.0 + np.exp(-h1))) * (x @ wg)) @ w2


def _dispatch(x, W1, Wg, W2, Ws1, Wsg, Ws2, gate_w, gate_b, biases):
    """Host-side routing + sharding. Returns (in_maps, core_idx, overflow, xf)."""
    x = np.asarray(x, dtype=np.float32)
    W1 = np.asarray(W1, dtype=np.float32)
    Wg = np.asarray(Wg, dtype=np.float32)
    W2 = np.asarray(W2, dtype=np.float32)
    Ws1 = np.asarray(Ws1, dtype=np.float32)
    Wsg = np.asarray(Wsg, dtype=np.float32)
    Ws2 = np.asarray(Ws2, dtype=np.float32)
    gate_w = np.asarray(gate_w, dtype=np.float32)
    gate_b = np.asarray(gate_b, dtype=np.float32)
    biases = np.asarray(biases, dtype=np.float32)

    xf = x.reshape(NTOK, D)

    # ---- routing (host): mirrors the reference math ----
    logits = xf @ gate_w + gate_b
    probas = _softmax_f32(logits)
    biased = probas + biases
    # jax.lax.top_k tie-break: lowest index first -> stable argsort of -biased
    topk = np.argsort(-biased, axis=-1, kind="stable")[:, :TOPK]
    tp = np.take_along_axis(probas, topk, axis=-1)
    tp = tp / tp.sum(axis=-1, keepdims=True)

    # ---- dispatch ----
    xbf = xf.astype(NP_BF16)
    w1bf = [W1[e].astype(NP_BF16) for e in range(E)]
    wgbf = [Wg[e].astype(NP_BF16) for e in range(E)]
    w2bf = [W2[e].astype(NP_BF16) for e in range(E)]
    ws1bf, wsgbf, ws2bf = (Ws1.astype(NP_BF16), Wsg.astype(NP_BF16),
                           Ws2.astype(NP_BF16))

    expert_tok = []   # token ids routed to expert e
    expert_wt = []    # their combine weights
    overflow = []     # (token, expert, weight) pairs beyond slot capacity
    for e in range(E):
        sel = (topk == e)
        rows = np.where(sel.any(axis=-1))[0]
        wts = (tp * sel).sum(axis=-1)[rows]
        if len(rows) > T_A:
            for t, w in zip(rows[T_A:], wts[T_A:]):
                overflow.append((int(t), e, float(w)))
            rows, wts = rows[:T_A], wts[:T_A]
        expert_tok.append(rows)
        expert_wt.append(wts.astype(np.float32))

    shared_chunks = [np.arange(T + T_B * i, T + T_B * (i + 1))
                     for i in range(E)]          # cores 0-6: T_B tokens each
    shared_chunks.append(np.arange(0, T))        # core 7: T tokens

    in_maps = []
    core_idx = []   # (idxA, nA, idxB, nB) for the combine step
    for c in range(N_CORES):
        if c < E:
            idx_a, wt_a = expert_tok[c], expert_wt[c]
            w1s, wgs, w2s = w1bf[c], wgbf[c], w2bf[c]
            idx_b = shared_chunks[c]
        else:
            idx_a = shared_chunks[c][:T_A]
            wt_a = np.ones(T_A, np.float32)
            w1s, wgs, w2s = ws1bf, wsgbf, ws2bf
            idx_b = shared_chunks[c][T_A:]
        n_a, n_b = len(idx_a), len(idx_b)

        xg = np.zeros((T, D), dtype=NP_BF16)
        xg[:n_a] = xbf[idx_a]
        xg[T_A:T_A + n_b] = xbf[idx_b]
        xt_c = np.ascontiguousarray(xg.T)

        s = np.zeros(SC_COLS * 128, np.float32)
        s[:n_a] = wt_a
        s[T_A:T_A + n_b] = 1.0
        sc_c = np.ascontiguousarray(s.reshape(SC_COLS, 128).T)

        in_maps.append({
            "xt": xt_c, "sc": sc_c,
            "w1a": w1s, "wga": wgs, "w2a": w2s,
            "w1b": ws1bf, "wgb": wsgbf, "w2b": ws2bf,
        })
        core_idx.append((idx_a, n_a, idx_b, n_b))

    return in_maps, core_idx, overflow, xf


def _combine(results, core_idx, overflow, xf, W1, Wg, W2):
    out = np.zeros((NTOK, D), np.float32)
    for c in range(N_CORES):
        yc = results[c]["y"]
        idx_a, n_a, idx_b, n_b = core_idx[c]
        out[idx_a] += yc[:n_a]
        out[idx_b] += yc[T_A:T_A + n_b]

    # correctness fallback if an expert exceeded slot capacity (never happens
    # for the benchmark distribution, but keeps the kernel total-correct)
    for t, e, w in overflow:
        out[t] += w * _np_ffn(xf[t:t + 1], np.asarray(W1[e], np.float32),
                              np.asarray(Wg[e], np.float32),
                              np.asarray(W2[e], np.float32))[0]

    return out.reshape(B, S, D)


def kernel(x, W1, Wg, W2, Ws1, Wsg, Ws2, gate_w, gate_b, biases):
    global LAST_RESULT
    in_maps, core_idx, overflow, xf = _dispatch(
        x, W1, Wg, W2, Ws1, Wsg, Ws2, gate_w, gate_b, biases)

    nc = _get_nc()
    res = run_bass_kernel_spmd(nc, in_maps, core_ids=list(range(N_CORES)))
    LAST_RESULT = res

    return _combine(res.results, core_idx, overflow, xf, W1, Wg, W2)
